# revision 1
# baseline (speedup 1.0000x reference)
"""2-layer GCN + classifier on 8 Trainium2 NeuronCores.

Strategy (graph/data parallel per sharding hint):
- Nodes sharded 8 ways by contiguous range (12500/core). Edges partitioned by
  dst shard on host, grouped by (dst-tile of 128, src-chunk of 32768), padded
  to 128-edge blocks. Self-loops appended as ordinary edges.
- Per GCN layer: each core builds its shard of the gather table
  T = dinv * (Z @ W) (bf16), AllGather -> full table in every core's HBM.
  Aggregation: dma_gather rows by src (int16 chunk-local idx), build one-hot
  dst-slot matrices via iota+is_equal, segment-sum via PSUM-accumulated bf16
  matmuls. Epilogue: dinv[dst]*agg + b, ELU, next W matmul (hi/lo split bf16
  pairs recover ~f32 weight precision), dinv pre-scale for next table.
- Classifier + log_softmax computed per node tile on device; host concatenates
  the 8 output shards.
"""
import sys

sys.path.insert(0, "/opt/trn_rl_repo")

import numpy as np
import ml_dtypes

import concourse.bacc as bacc
import concourse.tile as tile
from concourse import mybir
from concourse.bass_utils import run_bass_kernel_spmd

# ---------------- problem constants (hardcoded per task statement) ----------
N = 100000
E = 1600000
F_IN = 128
HID = 128
C_OUT = 40
NCORES = 8
NSH = N // NCORES          # 12500 nodes per core
P = 128
NT = (NSH + P - 1) // P    # 98 dst tiles per core (last has 84 rows)
NSH_PAD = NT * P           # 12544
CH = 32768                 # gather chunk rows (int16 idx limit)
NCHUNK = (N + CH - 1) // CH  # 4
TG = 12                    # dst tiles per tile-group (PSUM: 3 banks x 2 bufs)
NTG = (NT + TG - 1) // TG  # 9 tile groups

F32 = mybir.dt.float32
BF16 = mybir.dt.bfloat16
I16 = mybir.dt.int16


def _split_hi_lo(w):
    hi = w.astype(ml_dtypes.bfloat16)
    lo = (w - hi.astype(np.float32)).astype(ml_dtypes.bfloat16)
    return hi, lo


def _prep_host(x, edge_index, W0, b0, W1, b1, Wl, bl):
    """Shard + reorder edges; build all per-core device input arrays."""
    src = np.asarray(edge_index[0]).astype(np.int64)
    dst = np.asarray(edge_index[1]).astype(np.int64)
    loop = np.arange(N, dtype=np.int64)
    src2 = np.concatenate([src, loop])
    dst2 = np.concatenate([dst, loop])
    deg = np.bincount(dst2, minlength=N).astype(np.float32)  # = indeg + 1

    # ---- per-core edge grouping by (dst_tile, src_chunk) ----
    counts = np.zeros((NCORES, NT, NCHUNK), dtype=np.int64)
    grouped = []  # per core: (sorted_src_local int64, sorted_dstslot int64)
    core_of = dst2 // NSH
    for c in range(NCORES):
        sel = core_of == c
        es = src2[sel]
        ed = dst2[sel] - c * NSH
        tile_id = ed // P
        chunk_id = es // CH
        key = tile_id * NCHUNK + chunk_id
        order = np.argsort(key, kind="stable")
        es, ed, key = es[order], ed[order], key[order]
        cnt = np.bincount(key, minlength=NT * NCHUNK).reshape(NT, NCHUNK)
        counts[c] = cnt
        grouped.append((es, ed % P, cnt))

    bmax = np.ceil(counts.max(axis=0) / P).astype(np.int64)  # [NT, NCHUNK] blocks
    nblk_tk = bmax  # blocks per (t, k), shared across cores

    # flatten order: for tg: for k: for t in tg: blocks
    # Build per-core gidx (chunk-local src, wrapped) and dstslot streams.
    tot_blocks = 0
    for g in range(NTG):
        tiles = range(g * TG, min((g + 1) * TG, NT))
        for k in range(NCHUNK):
            for t in tiles:
                tot_blocks += int(nblk_tk[t][k])
    tot_slots = tot_blocks * P

    gidx_all = np.zeros((NCORES, 128, tot_slots // 16), dtype=np.int16)
    dsl_all = np.full((NCORES, 128, tot_blocks), -1.0, dtype=np.float32)

    for c in range(NCORES):
        es, slots, cnt = grouped[c]
        starts = np.zeros(NT * NCHUNK + 1, dtype=np.int64)
        np.cumsum(cnt.reshape(-1), out=starts[1:])
        gcol = 0  # gidx column offset (16-wide units)
        bcol = 0  # dstslot block column offset
        for g in range(NTG):
            tiles = range(g * TG, min((g + 1) * TG, NT))
            for k in range(NCHUNK):
                for t in tiles:
                    nb = int(nblk_tk[t][k])
                    if nb == 0:
                        continue
                    a, b = starts[t * NCHUNK + k], starts[t * NCHUNK + k + 1]
                    n = b - a
                    pad = nb * P - n
                    sl = np.concatenate(
                        [es[a:b] - k * CH, np.zeros(pad, dtype=np.int64)]
                    ).astype(np.int16)
                    ds = np.concatenate(
                        [slots[a:b], np.full(pad, -1, dtype=np.int64)]
                    ).astype(np.float32)
                    # wrap idx: slot i -> [i%16, i//16]
                    gidx_all[c, :16, gcol : gcol + nb * 8] = sl.reshape(-1, 16).T
                    dsl_all[c, :, bcol : bcol + nb] = ds.reshape(nb, P).T
                    gcol += nb * 8
                    bcol += nb
        gidx_all[c] = np.tile(gidx_all[c, :16], (8, 1))

    # degree layouts
    deg_col = np.ones((NCORES, 128, NT), dtype=np.float32)
    deg_row = np.ones((NCORES, 1, NSH_PAD), dtype=np.float32)
    for c in range(NCORES):
        d = deg[c * NSH : (c + 1) * NSH]
        dp = np.concatenate([d, np.ones(NSH_PAD - NSH, dtype=np.float32)])
        deg_col[c] = dp.reshape(NT, P).T
        deg_row[c, 0] = dp

    # x transposed shards, hi/lo bf16
    xT_hi = np.zeros((NCORES, 128, NSH_PAD), dtype=ml_dtypes.bfloat16)
    xT_lo = np.zeros((NCORES, 128, NSH_PAD), dtype=ml_dtypes.bfloat16)
    for c in range(NCORES):
        xs = np.asarray(x[c * NSH : (c + 1) * NSH]).astype(np.float32).T  # [128, NSH]
        hi, lo = _split_hi_lo(xs)
        xT_hi[c, :, :NSH] = hi
        xT_lo[c, :, :NSH] = lo

    W0h, W0l = _split_hi_lo(np.asarray(W0, dtype=np.float32))
    W1h, W1l = _split_hi_lo(np.asarray(W1, dtype=np.float32))
    Wlh, Wll = _split_hi_lo(np.asarray(Wl, dtype=np.float32))
    b0c = np.asarray(b0, dtype=np.float32).reshape(128, 1)
    b1c = np.asarray(b1, dtype=np.float32).reshape(128, 1)
    blb = np.tile(np.asarray(bl, dtype=np.float32).reshape(1, C_OUT), (128, 1))

    in_maps = []
    for c in range(NCORES):
        in_maps.append(
            {
                "xT_hi": xT_hi[c],
                "xT_lo": xT_lo[c],
                "gidx": gidx_all[c],
                "dsl": dsl_all[c],
                "deg_col": deg_col[c],
                "deg_row": deg_row[c],
                "W0h": W0h, "W0l": W0l,
                "W1h": W1h, "W1l": W1l,
                "Wlh": Wlh, "Wll": Wll,
                "b0c": b0c, "b1c": b1c, "blb": blb,
            }
        )
    return in_maps, nblk_tk, tot_blocks, tot_slots


def _build_program(nblk_tk, tot_blocks, tot_slots):
    nc = bacc.Bacc(num_devices=NCORES)
    xT_hi = nc.declare_dram_parameter("xT_hi", [128, NSH_PAD], BF16, isOutput=False)
    xT_lo = nc.declare_dram_parameter("xT_lo", [128, NSH_PAD], BF16, isOutput=False)
    gidx = nc.declare_dram_parameter("gidx", [128, tot_slots // 16], I16, isOutput=False)
    dsl = nc.declare_dram_parameter("dsl", [128, tot_blocks], F32, isOutput=False)
    deg_col = nc.declare_dram_parameter("deg_col", [128, NT], F32, isOutput=False)
    deg_row = nc.declare_dram_parameter("deg_row", [1, NSH_PAD], F32, isOutput=False)
    W0h = nc.declare_dram_parameter("W0h", [128, HID], BF16, isOutput=False)
    W0l = nc.declare_dram_parameter("W0l", [128, HID], BF16, isOutput=False)
    W1h = nc.declare_dram_parameter("W1h", [128, HID], BF16, isOutput=False)
    W1l = nc.declare_dram_parameter("W1l", [128, HID], BF16, isOutput=False)
    Wlh = nc.declare_dram_parameter("Wlh", [128, C_OUT], BF16, isOutput=False)
    Wll = nc.declare_dram_parameter("Wll", [128, C_OUT], BF16, isOutput=False)
    b0c = nc.declare_dram_parameter("b0c", [128, 1], F32, isOutput=False)
    b1c = nc.declare_dram_parameter("b1c", [128, 1], F32, isOutput=False)
    blb = nc.declare_dram_parameter("blb", [128, C_OUT], F32, isOutput=False)
    out_ext = nc.declare_dram_parameter("out", [NSH, C_OUT], F32, isOutput=True)

    t1_shard = nc.dram_tensor("t1_shard", [NSH, HID], BF16)
    t2_shard = nc.dram_tensor("t2_shard", [NSH, HID], BF16)
    T1_full = nc.dram_tensor("T1_full", [N, HID], BF16, addr_space="Shared")
    T2_full = nc.dram_tensor("T2_full", [N, HID], BF16, addr_space="Shared")

    # per-(tg,k) slot counts and offsets
    sgk = np.zeros((NTG, NCHUNK), dtype=np.int64)
    for g in range(NTG):
        tiles = range(g * TG, min((g + 1) * TG, NT))
        for k in range(NCHUNK):
            sgk[g][k] = P * sum(int(nblk_tk[t][k]) for t in tiles)
    max_gk_blocks = int(sgk.max()) // P

    from contextlib import ExitStack
    with tile.TileContext(nc) as tc, ExitStack() as es:
        cpool = es.enter_context(tc.tile_pool(name="const", bufs=1))
        xpool = es.enter_context(tc.tile_pool(name="xp", bufs=3))
        gpool = es.enter_context(tc.tile_pool(name="gp", bufs=2))
        ipool = es.enter_context(tc.tile_pool(name="ip", bufs=2))
        dpool = es.enter_context(tc.tile_pool(name="dp", bufs=2))
        spool = es.enter_context(tc.tile_pool(name="sp", bufs=6))
        zpool = es.enter_context(tc.tile_pool(name="zp", bufs=2))
        opool = es.enter_context(tc.tile_pool(name="op", bufs=2))
        apsum = es.enter_context(tc.tile_pool(name="apsum", bufs=2, space="PSUM"))
        wpsum = es.enter_context(tc.tile_pool(name="wpsum", bufs=2, space="PSUM"))

        # ---- constants ----
        iota_t = cpool.tile([P, P], BF16, tag="iota")
        nc.gpsimd.iota(iota_t[:], pattern=[[1, P]], base=0, channel_multiplier=0,
                       allow_small_or_imprecise_dtypes=True)
        w0h_t = cpool.tile([128, HID], BF16, tag="w0h")
        w0l_t = cpool.tile([128, HID], BF16, tag="w0l")
        w1h_t = cpool.tile([128, HID], BF16, tag="w1h")
        w1l_t = cpool.tile([128, HID], BF16, tag="w1l")
        wlh_t = cpool.tile([128, C_OUT], BF16, tag="wlh")
        wll_t = cpool.tile([128, C_OUT], BF16, tag="wll")
        b0_t = cpool.tile([128, 1], F32, tag="b0")
        b1_t = cpool.tile([128, 1], F32, tag="b1")
        blb_t = cpool.tile([128, C_OUT], F32, tag="blb")
        for tt, ext in [(w0h_t, W0h), (w0l_t, W0l), (w1h_t, W1h), (w1l_t, W1l),
                        (wlh_t, Wlh), (wll_t, Wll), (b0_t, b0c), (b1_t, b1c),
                        (blb_t, blb)]:
            nc.sync.dma_start(out=tt[:], in_=ext[:, :])

        # ---- dinv (column and broadcast layouts) ----
        dcol_raw = cpool.tile([128, NT], F32, tag="dcolr")
        nc.sync.dma_start(out=dcol_raw[:], in_=deg_col[:, :])
        dcol_s = cpool.tile([128, NT], F32, tag="dcols")
        nc.scalar.activation(dcol_s[:], dcol_raw[:], mybir.ActivationFunctionType.Sqrt)
        dinv_col = cpool.tile([128, NT], F32, tag="dcol")
        nc.vector.reciprocal(dinv_col[:], dcol_s[:])

        # dinv broadcast tile: row0=deg -> row1=sqrt -> row0=1/sqrt -> double up
        dinv_b = cpool.tile([128, NSH_PAD], F32, tag="dinvb")
        nc.sync.dma_start(out=dinv_b[:1, :], in_=deg_row[:, :])
        nc.scalar.activation(dinv_b[32:33, :], dinv_b[:1, :],
                             mybir.ActivationFunctionType.Sqrt)
        nc.vector.reciprocal(dinv_b[:1, :], dinv_b[32:33, :])
        rows_done = 1
        while rows_done < 128:
            n = min(rows_done, 128 - rows_done)
            nc.sync.dma_start(out=dinv_b[rows_done : rows_done + n, :],
                              in_=dinv_b[:n, :])
            rows_done += n

        # ---- phase 1: T1 shard = dinv * (X @ W0) ----
        for t in range(NT):
            rows = min(P, NSH - t * P)
            xh = xpool.tile([128, P], BF16, tag="xh")
            xl = xpool.tile([128, P], BF16, tag="xl")
            nc.sync.dma_start(out=xh[:], in_=xT_hi[:, t * P : (t + 1) * P])
            nc.sync.dma_start(out=xl[:], in_=xT_lo[:, t * P : (t + 1) * P])
            ps = wpsum.tile([P, HID], F32, tag="wps", space="PSUM")
            nc.tensor.matmul(out=ps[:], lhsT=xh[:], rhs=w0h_t[:], start=True, stop=False)
            nc.tensor.matmul(out=ps[:], lhsT=xh[:], rhs=w0l_t[:], start=False, stop=False)
            nc.tensor.matmul(out=ps[:], lhsT=xl[:], rhs=w0h_t[:], start=False, stop=True)
            tb = opool.tile([P, HID], BF16, tag="tb")
            nc.vector.tensor_scalar(out=tb[:], in0=ps[:], scalar1=dinv_col[:, t : t + 1],
                                    scalar2=None, op0=mybir.AluOpType.mult)
            nc.sync.dma_start(out=t1_shard[t * P : t * P + rows, :], in_=tb[:rows, :])

        # ---- allgather T1 ----
        nc.gpsimd.collective_compute(
            "AllGather", mybir.AluOpType.bypass,
            replica_groups=[list(range(NCORES))],
            ins=[t1_shard[:].opt()], outs=[T1_full[:].opt()],
        )

        # ---- aggregation layers ----
        def agg_layer(T_full, layer):
            gcol = [0]  # running gidx column offset (16-wide)
            bcol = [0]  # running dstslot block column
            for g in range(NTG):
                tiles = list(range(g * TG, min((g + 1) * TG, NT)))
                # per-(t) first/last chunk with blocks, for start/stop flags
                first_k = {}
                last_k = {}
                for t in tiles:
                    ks = [k for k in range(NCHUNK) if nblk_tk[t][k] > 0]
                    first_k[t], last_k[t] = ks[0], ks[-1]
                # allocate agg psum: bank tiles of [128, 512]; tile t -> slot
                nbank = (len(tiles) + 3) // 4
                banks = [apsum.tile([P, 512], F32, tag=f"agg{i}", space="PSUM",
                                    name=f"aggbank{i}")
                         for i in range(nbank)]

                def agg_ap(ti):
                    i = tiles.index(ti)
                    return banks[i // 4][:, (i % 4) * P : (i % 4 + 1) * P]

                # dstslot slab for this tile group
                nb_tg = sum(int(nblk_tk[t][k]) for t in tiles for k in range(NCHUNK))
                dst_t = dpool.tile([128, nb_tg], F32, tag="dsl")
                nc.sync.dma_start(out=dst_t[:], in_=dsl[:, bcol[0] : bcol[0] + nb_tg])

                dcol_off = 0
                for k in range(NCHUNK):
                    s_gk = int(sgk[g][k])
                    if s_gk == 0:
                        continue
                    kend = min((k + 1) * CH, N)
                    idxt = ipool.tile([128, max(int(sgk.max()) // 16, 16)], I16, tag="gidx")
                    nc.sync.dma_start(out=idxt[:, : s_gk // 16],
                                      in_=gidx[:, gcol[0] : gcol[0] + s_gk // 16])
                    gbuf = gpool.tile([P, max_gk_blocks, P], BF16, tag="gath")
                    nblk_gk = s_gk // P
                    nc.gpsimd.dma_gather(
                        gbuf[:, :nblk_gk, :], T_full[k * CH : kend, :],
                        idxt[:, : s_gk // 16], s_gk, s_gk, HID,
                        single_packet=False,
                    )
                    gcol[0] += s_gk // 16
                    boff = 0
                    for t in tiles:
                        nb = int(nblk_tk[t][k])
                        for b in range(nb):
                            s_t = spool.tile([P, P], BF16, tag="onehot")
                            nc.vector.tensor_scalar(
                                out=s_t[:], in0=iota_t[:],
                                scalar1=dst_t[:, dcol_off : dcol_off + 1],
                                scalar2=None, op0=mybir.AluOpType.is_equal,
                            )
                            nc.tensor.matmul(
                                out=agg_ap(t), lhsT=gbuf[:, boff, :], rhs=s_t[:],
                                start=(k == first_k[t] and b == 0),
                                stop=(k == last_k[t] and b == nb - 1),
                                skip_group_check=True,
                            )
                            boff += 1
                            dcol_off += 1
                bcol[0] += nb_tg

                # ---- epilogue per tile ----
                for t in tiles:
                    rows = min(P, NSH - t * P)
                    bias = b0_t if layer == 1 else b1_t
                    u = zpool.tile([P, P], F32, tag="u")
                    nc.vector.tensor_tensor(
                        out=u[:], in0=agg_ap(t),
                        in1=dinv_b[:, t * P : (t + 1) * P],
                        op=mybir.AluOpType.mult)
                    m = zpool.tile([P, P], F32, tag="m")
                    nc.vector.tensor_scalar(out=m[:], in0=u[:], scalar1=bias[:],
                                            scalar2=0.0, op0=mybir.AluOpType.add,
                                            op1=mybir.AluOpType.min)
                    pmax = zpool.tile([P, P], F32, tag="pmax")
                    nc.vector.tensor_scalar(out=pmax[:], in0=u[:], scalar1=bias[:],
                                            scalar2=0.0, op0=mybir.AluOpType.add,
                                            op1=mybir.AluOpType.max)
                    e = zpool.tile([P, P], F32, tag="e")
                    nc.scalar.activation(e[:], m[:], mybir.ActivationFunctionType.Exp)
                    zs = zpool.tile([P, P], F32, tag="zs")
                    nc.vector.tensor_tensor(out=zs[:], in0=pmax[:], in1=e[:],
                                            op=mybir.AluOpType.add)
                    zb = zpool.tile([P, P], BF16, tag="zb")
                    nc.vector.tensor_scalar(out=zb[:], in0=zs[:], scalar1=-1.0,
                                            scalar2=None, op0=mybir.AluOpType.add)
                    if layer == 1:
                        ps2 = wpsum.tile([P, HID], F32, tag="wps", space="PSUM")
                        nc.tensor.matmul(out=ps2[:], lhsT=zb[:], rhs=w1h_t[:],
                                         start=True, stop=False)
                        nc.tensor.matmul(out=ps2[:], lhsT=zb[:], rhs=w1l_t[:],
                                         start=False, stop=True)
                        t2b = opool.tile([P, HID], BF16, tag="tb")
                        nc.vector.tensor_scalar(out=t2b[:], in0=ps2[:],
                                                scalar1=dinv_col[:, t : t + 1],
                                                scalar2=None, op0=mybir.AluOpType.mult)
                        nc.sync.dma_start(out=t2_shard[t * P : t * P + rows, :],
                                          in_=t2b[:rows, :])
                    else:
                        ps3 = wpsum.tile([P, HID], F32, tag="wps", space="PSUM")
                        nc.tensor.matmul(out=ps3[:, :C_OUT], lhsT=zb[:], rhs=wlh_t[:],
                                         start=True, stop=False)
                        nc.tensor.matmul(out=ps3[:, :C_OUT], lhsT=zb[:], rhs=wll_t[:],
                                         start=False, stop=True)
                        lg = opool.tile([P, C_OUT], F32, tag="lg")
                        nc.vector.tensor_tensor(out=lg[:], in0=ps3[:, :C_OUT],
                                                in1=blb_t[:], op=mybir.AluOpType.add)
                        mx = opool.tile([P, 1], F32, tag="mx")
                        nc.vector.tensor_reduce(out=mx[:], in_=lg[:],
                                                axis=mybir.AxisListType.X,
                                                op=mybir.AluOpType.max)
                        sh = opool.tile([P, C_OUT], F32, tag="sh")
                        nc.vector.tensor_scalar(out=sh[:], in0=lg[:], scalar1=mx[:],
                                                scalar2=None,
                                                op0=mybir.AluOpType.subtract)
                        ex = opool.tile([P, C_OUT], F32, tag="ex")
                        sm = opool.tile([P, 1], F32, tag="sm")
                        nc.scalar.activation(ex[:], sh[:],
                                             mybir.ActivationFunctionType.Exp,
                                             accum_out=sm[:])
                        ln = opool.tile([P, 1], F32, tag="ln")
                        nc.scalar.activation(ln[:], sm[:],
                                             mybir.ActivationFunctionType.Ln)
                        res = opool.tile([P, C_OUT], F32, tag="res")
                        nc.vector.tensor_scalar(out=res[:], in0=sh[:], scalar1=ln[:],
                                                scalar2=None,
                                                op0=mybir.AluOpType.subtract)
                        nc.sync.dma_start(out=out_ext[t * P : t * P + rows, :],
                                          in_=res[:rows, :])

        agg_layer(T1_full, 1)
        nc.gpsimd.collective_compute(
            "AllGather", mybir.AluOpType.bypass,
            replica_groups=[list(range(NCORES))],
            ins=[t2_shard[:].opt()], outs=[T2_full[:].opt()],
        )
        agg_layer(T2_full, 2)

    nc.finalize()
    return nc


_CACHE = {}


def kernel(**inputs):
    in_maps, nblk_tk, tot_blocks, tot_slots = _prep_host(
        inputs["x"], inputs["edge_index"], inputs["W0"], inputs["b0"],
        inputs["W1"], inputs["b1"], inputs["Wl"], inputs["bl"])
    key = (tot_blocks, tot_slots, nblk_tk.tobytes())
    if key not in _CACHE:
        _CACHE[key] = _build_program(nblk_tk, tot_blocks, tot_slots)
    nc = _CACHE[key]
    trace = bool(int(__import__("os").environ.get("KERNEL_TRACE", "0")))
    res = run_bass_kernel_spmd(nc, in_maps, list(range(NCORES)), trace=trace)
    kernel.last_results = res
    out = np.concatenate([res.results[c]["out"] for c in range(NCORES)], axis=0)
    return out.astype(np.float32)



# revision 2
# speedup vs baseline: 1.0373x; 1.0373x over previous
"""2-layer GCN + classifier on 8 Trainium2 NeuronCores — v3.

v2 + gather-padding trim and pipeline gap removal:
- Edge blocks packed per (tile-group, chunk) run instead of per (tile, chunk):
  ~2% padding instead of ~19%. Blocks may straddle dst-tile boundaries; the
  matmul schedule is the UNION over cores of (block, tile) pairs, with
  per-core one-hot columns zeroing non-local lanes.
- Whole-layer gather index stream loaded in ONE DMA up front (gathers never
  wait on index loads).
- T-shard tiles pinned in SBUF from the moment they are produced, so the
  self-loop diagonal matmuls need no reloads.
- One-hot slabs + sdiag slabs loaded on the Activation HWDGE queue, keeping
  the SP queue short.
"""
import sys

sys.path.insert(0, "/opt/trn_rl_repo")

import numpy as np
import ml_dtypes

import concourse.bacc as bacc
import concourse.tile as tile
from concourse import mybir
from concourse.bass_utils import run_bass_kernel_spmd

N = 100000
E = 1600000
F_IN = 128
HID = 128
C_OUT = 40
NCORES = 8
NSH = N // NCORES          # 12500
P = 128
NT = (NSH + P - 1) // P    # 98
NSH_PAD = NT * P           # 12544
CH = 32768
NCHUNK = (N + CH - 1) // CH  # 4
TG = 12
NTG = (NT + TG - 1) // TG  # 9
NRUN = NTG * NCHUNK

F32 = mybir.dt.float32
BF16 = mybir.dt.bfloat16
I16 = mybir.dt.int16


def _split_hi_lo(w):
    hi = w.astype(ml_dtypes.bfloat16)
    lo = (w - hi.astype(np.float32)).astype(ml_dtypes.bfloat16)
    return hi, lo


def _prep_host(x, edge_index, W0, b0, W1, b1, Wl, bl):
    src = np.asarray(edge_index[0]).astype(np.int64)
    dst = np.asarray(edge_index[1]).astype(np.int64)
    loop = np.arange(N, dtype=np.int64)
    deg = np.bincount(np.concatenate([dst, loop]), minlength=N).astype(np.float64)
    dinv = 1.0 / np.sqrt(deg)
    wnorm = (dinv[src] * dinv[dst]).astype(np.float32)

    sdiag_all = np.zeros((NCORES, 128, NSH_PAD), dtype=ml_dtypes.bfloat16)
    for c in range(NCORES):
        nodes = np.arange(NSH, dtype=np.int64)
        sdiag_all[c][nodes % P, nodes] = (1.0 / deg[c * NSH + nodes]).astype(
            np.float32
        )

    core_of = dst // NSH
    per_core = []
    len_rc = np.zeros((NCORES, NRUN), dtype=np.int64)
    for c in range(NCORES):
        sel = core_of == c
        es = src[sel]
        ed = dst[sel] - c * NSH
        w = wnorm[sel]
        t_id = ed // P
        k_id = es // CH
        g_id = t_id // TG
        run = g_id * NCHUNK + k_id
        key = run * NT + t_id
        order = np.argsort(key, kind="stable")
        es, ed, w, run, t_id = (es[order], ed[order], w[order],
                                run[order], t_id[order])
        len_rc[c] = np.bincount(run, minlength=NRUN)
        per_core.append((es, ed % P, w, run, t_id, k_id[order]))

    nblk_r = np.ceil(len_rc.max(axis=0) / P).astype(np.int64)   # [NRUN]
    blkstart_r = np.zeros(NRUN + 1, dtype=np.int64)
    np.cumsum(nblk_r, out=blkstart_r[1:])
    tot_blocks = int(blkstart_r[-1])
    tot_slots = tot_blocks * P

    # per-core (run, block, tile) triples -> union schedule
    MAXJ = int(nblk_r.max()) + 1
    codes_per_core = []
    pos_per_core = []
    for c in range(NCORES):
        es, slot, w, run, t_id, k_s = per_core[c]
        runstart = np.zeros(NRUN + 1, dtype=np.int64)
        np.cumsum(len_rc[c], out=runstart[1:])
        pos = np.arange(len(es), dtype=np.int64) - runstart[run]
        j = pos // P
        code = (run * MAXJ + j) * NT + t_id
        codes_per_core.append(code)
        pos_per_core.append(pos)

    union = np.unique(np.concatenate(codes_per_core))   # sorted (run, j, t)
    tot_sched = len(union)
    u_run = union // (MAXJ * NT)
    u_j = (union // NT) % MAXJ
    u_t = union % NT
    # sched entries for run r occupy ohcols [schedstart_r[r], schedstart_r[r+1])
    schedstart_r = np.searchsorted(u_run, np.arange(NRUN + 1))

    # stop flags: per group g, last sched entry (over k asc, entry asc) per tile
    stop_flag = np.zeros(tot_sched, dtype=bool)
    t_has = [set() for _ in range(NTG)]
    for g in range(NTG):
        last_for_t = {}
        for k in range(NCHUNK):
            r = g * NCHUNK + k
            for s in range(schedstart_r[r], schedstart_r[r + 1]):
                last_for_t[int(u_t[s])] = s
                t_has[g].add(int(u_t[s]))
        for t, s in last_for_t.items():
            stop_flag[s] = True

    sched = []   # per run: list of (j, t, ohcol, stop)
    for r in range(NRUN):
        ent = [(int(u_j[s]), int(u_t[s]), int(s), bool(stop_flag[s]))
               for s in range(schedstart_r[r], schedstart_r[r + 1])]
        sched.append(ent)

    gidx_all = np.zeros((NCORES, 128, tot_slots // 16), dtype=np.int16)
    oh_all = np.zeros((NCORES, 128, tot_sched * P), dtype=ml_dtypes.bfloat16)
    for c in range(NCORES):
        es, slot, w, run, t_id, k_s = per_core[c]
        pos = pos_per_core[c]
        idxflat = np.zeros(tot_slots, dtype=np.int16)
        idxflat[blkstart_r[run] * P + pos] = (es - k_s * CH).astype(np.int16)
        gidx_all[c] = np.tile(idxflat.reshape(-1, 16).T, (8, 1))
        ohcol = np.searchsorted(union, codes_per_core[c])
        oh_all[c][pos % P, ohcol * P + slot] = w

    xT_hi = np.zeros((NCORES, 128, NSH_PAD), dtype=ml_dtypes.bfloat16)
    xT_lo = np.zeros((NCORES, 128, NSH_PAD), dtype=ml_dtypes.bfloat16)
    for c in range(NCORES):
        xs = np.asarray(x[c * NSH : (c + 1) * NSH]).astype(np.float32).T
        hi, lo = _split_hi_lo(xs)
        xT_hi[c, :, :NSH] = hi
        xT_lo[c, :, :NSH] = lo

    W0h, W0l = _split_hi_lo(np.asarray(W0, dtype=np.float32))
    W1h, W1l = _split_hi_lo(np.asarray(W1, dtype=np.float32))
    Wlh, Wll = _split_hi_lo(np.asarray(Wl, dtype=np.float32))
    b0c = np.asarray(b0, dtype=np.float32).reshape(128, 1)
    b1c = np.asarray(b1, dtype=np.float32).reshape(128, 1)
    blrow = np.asarray(bl, dtype=np.float32).reshape(1, C_OUT).astype(
        ml_dtypes.bfloat16)

    in_maps = []
    for c in range(NCORES):
        in_maps.append(
            {
                "xT_hi": xT_hi[c], "xT_lo": xT_lo[c],
                "gidx": gidx_all[c], "oh": oh_all[c], "sdiag": sdiag_all[c],
                "W0h": W0h, "W0l": W0l,
                "W1h": W1h, "W1l": W1l,
                "nW1h": -W1h, "nW1l": -W1l,
                "Wlh": Wlh, "Wll": Wll,
                "nWlh": -Wlh, "nWll": -Wll,
                "b0c": b0c, "b1c": b1c, "blrow": blrow,
            }
        )
    meta = dict(sched=sched, nblk_r=nblk_r, blkstart_r=blkstart_r,
                schedstart_r=schedstart_r, t_has=t_has,
                tot_blocks=tot_blocks, tot_slots=tot_slots,
                tot_sched=tot_sched)
    return in_maps, meta


def _build_program(meta):
    sched = meta["sched"]
    nblk_r = meta["nblk_r"]
    blkstart_r = meta["blkstart_r"]
    schedstart_r = meta["schedstart_r"]
    t_has = meta["t_has"]
    tot_blocks = meta["tot_blocks"]
    tot_slots = meta["tot_slots"]
    tot_sched = meta["tot_sched"]
    max_blk = int(nblk_r.max())
    max_sched = int(max(schedstart_r[r + 1] - schedstart_r[r]
                        for r in range(NRUN)))

    nc = bacc.Bacc(num_devices=NCORES)
    xT_hi = nc.declare_dram_parameter("xT_hi", [128, NSH_PAD], BF16, isOutput=False)
    xT_lo = nc.declare_dram_parameter("xT_lo", [128, NSH_PAD], BF16, isOutput=False)
    gidx = nc.declare_dram_parameter("gidx", [128, tot_slots // 16], I16,
                                     isOutput=False)
    ohp = nc.declare_dram_parameter("oh", [128, tot_sched * P], BF16,
                                    isOutput=False)
    sdiag = nc.declare_dram_parameter("sdiag", [128, NSH_PAD], BF16,
                                      isOutput=False)
    W0h = nc.declare_dram_parameter("W0h", [128, HID], BF16, isOutput=False)
    W0l = nc.declare_dram_parameter("W0l", [128, HID], BF16, isOutput=False)
    W1h = nc.declare_dram_parameter("W1h", [128, HID], BF16, isOutput=False)
    W1l = nc.declare_dram_parameter("W1l", [128, HID], BF16, isOutput=False)
    nW1h = nc.declare_dram_parameter("nW1h", [128, HID], BF16, isOutput=False)
    nW1l = nc.declare_dram_parameter("nW1l", [128, HID], BF16, isOutput=False)
    Wlh = nc.declare_dram_parameter("Wlh", [128, C_OUT], BF16, isOutput=False)
    Wll = nc.declare_dram_parameter("Wll", [128, C_OUT], BF16, isOutput=False)
    nWlh = nc.declare_dram_parameter("nWlh", [128, C_OUT], BF16, isOutput=False)
    nWll = nc.declare_dram_parameter("nWll", [128, C_OUT], BF16, isOutput=False)
    b0c = nc.declare_dram_parameter("b0c", [128, 1], F32, isOutput=False)
    b1c = nc.declare_dram_parameter("b1c", [128, 1], F32, isOutput=False)
    blrow = nc.declare_dram_parameter("blrow", [1, C_OUT], BF16, isOutput=False)
    out_ext = nc.declare_dram_parameter("out", [NSH, C_OUT], F32, isOutput=True)

    t1_shard = nc.dram_tensor("t1_shard", [NSH, HID], BF16)
    t2_shard = nc.dram_tensor("t2_shard", [NSH, HID], BF16)
    T1_full = nc.dram_tensor("T1_full", [N, HID], BF16, addr_space="Shared")
    T2_full = nc.dram_tensor("T2_full", [N, HID], BF16, addr_space="Shared")

    AF = mybir.ActivationFunctionType

    from contextlib import ExitStack
    with tile.TileContext(nc) as tc, ExitStack() as es:
        cpool = es.enter_context(tc.tile_pool(name="const", bufs=1))
        tpool = es.enter_context(tc.tile_pool(name="tsh", bufs=1))
        xpool = es.enter_context(tc.tile_pool(name="xp", bufs=3))
        gpool = es.enter_context(tc.tile_pool(name="gp", bufs=2))
        opool = es.enter_context(tc.tile_pool(name="ohp", bufs=2))
        dpool = es.enter_context(tc.tile_pool(name="dg", bufs=2))
        zpool = es.enter_context(tc.tile_pool(name="zp", bufs=4))
        apsum = es.enter_context(tc.tile_pool(name="apsum", bufs=2, space="PSUM"))
        wpsum = es.enter_context(tc.tile_pool(name="wpsum", bufs=2, space="PSUM"))

        # ---- constants ----
        w0h_t = cpool.tile([128, HID], BF16, tag="w0h")
        w0l_t = cpool.tile([128, HID], BF16, tag="w0l")
        w1h_t = cpool.tile([128, HID], BF16, tag="w1h")
        w1l_t = cpool.tile([128, HID], BF16, tag="w1l")
        nw1h_t = cpool.tile([128, HID], BF16, tag="nw1h")
        nw1l_t = cpool.tile([128, HID], BF16, tag="nw1l")
        wlh_t = cpool.tile([128, C_OUT], BF16, tag="wlh")
        wll_t = cpool.tile([128, C_OUT], BF16, tag="wll")
        nwlh_t = cpool.tile([128, C_OUT], BF16, tag="nwlh")
        nwll_t = cpool.tile([128, C_OUT], BF16, tag="nwll")
        b0_t = cpool.tile([128, 1], F32, tag="b0")
        b1_t = cpool.tile([128, 1], F32, tag="b1")
        blr_t = cpool.tile([1, C_OUT], BF16, tag="blr")
        for tt, ext in [(w0h_t, W0h), (w0l_t, W0l), (w1h_t, W1h), (w1l_t, W1l),
                        (nw1h_t, nW1h), (nw1l_t, nW1l),
                        (wlh_t, Wlh), (wll_t, Wll), (nwlh_t, nWlh),
                        (nwll_t, nWll), (b0_t, b0c), (b1_t, b1c)]:
            nc.sync.dma_start(out=tt[:], in_=ext[:, :])
        nc.sync.dma_start(out=blr_t[:], in_=blrow[:, :])
        ones_t = cpool.tile([1, P], BF16, tag="ones")
        nc.vector.memset(ones_t[:], 1.0)

        # whole-program gather index stream (shared by both layers)
        gidx_t = cpool.tile([128, tot_slots // 16], I16, tag="gidx")
        nc.sync.dma_start(out=gidx_t[:], in_=gidx[:, :])

        # pinned T-shard tiles
        t1_tiles = [tpool.tile([P, HID], BF16, tag=f"t1_{t}", name=f"t1_{t}")
                    for t in range(NT)]
        t2_tiles = [tpool.tile([P, HID], BF16, tag=f"t2_{t}", name=f"t2_{t}")
                    for t in range(NT)]

        # ---- phase 1: T1 shard = X @ W0 ----
        for t in range(NT):
            rows = min(P, NSH - t * P)
            xh = xpool.tile([128, P], BF16, tag="xh")
            xl = xpool.tile([128, P], BF16, tag="xl")
            nc.sync.dma_start(out=xh[:], in_=xT_hi[:, t * P : (t + 1) * P])
            nc.sync.dma_start(out=xl[:], in_=xT_lo[:, t * P : (t + 1) * P])
            ps = wpsum.tile([P, HID], F32, tag="wps", space="PSUM")
            nc.tensor.matmul(out=ps[:], lhsT=xh[:], rhs=w0h_t[:], start=True, stop=False)
            nc.tensor.matmul(out=ps[:], lhsT=xh[:], rhs=w0l_t[:], start=False, stop=False)
            nc.tensor.matmul(out=ps[:], lhsT=xl[:], rhs=w0h_t[:], start=False, stop=True)
            tb = t1_tiles[t]
            nc.scalar.activation(tb[:], ps[:], AF.Copy)
            nc.sync.dma_start(out=t1_shard[t * P : t * P + rows, :], in_=tb[:rows, :])

        nc.gpsimd.collective_compute(
            "AllGather", mybir.AluOpType.bypass,
            replica_groups=[list(range(NCORES))],
            ins=[t1_shard[:].opt()], outs=[T1_full[:].opt()],
        )

        def agg_layer(T_full, t_tiles, out_tiles, layer):
            bias_t = b0_t if layer == 1 else b1_t
            for g in range(NTG):
                tiles = list(range(g * TG, min((g + 1) * TG, NT)))
                nbank = (len(tiles) + 3) // 4
                banks = [apsum.tile([P, 512], F32, tag=f"agg{i}", space="PSUM",
                                    name=f"aggbank{i}")
                         for i in range(nbank)]

                def agg_ap(ti):
                    i = tiles.index(ti)
                    return banks[i // 4][:, (i % 4) * P : (i % 4 + 1) * P]

                # sdiag slab for this group (Act HWDGE queue)
                sds = dpool.tile([128, TG * P], BF16, tag="sds")
                nc.scalar.dma_start(
                    out=sds[:, : len(tiles) * P],
                    in_=sdiag[:, g * TG * P : g * TG * P + len(tiles) * P])

                # self-loop diagonal opens each tile's PSUM group
                for i, t in enumerate(tiles):
                    rows = min(P, NSH - t * P)
                    nc.tensor.matmul(out=agg_ap(t), lhsT=t_tiles[t][:rows, :],
                                     rhs=sds[:rows, i * P : (i + 1) * P],
                                     start=True, stop=(t not in t_has[g]),
                                     skip_group_check=True)

                for k in range(NCHUNK):
                    r = g * NCHUNK + k
                    nblk = int(nblk_r[r])
                    if nblk == 0:
                        continue
                    s_gk = nblk * P
                    nsched = int(schedstart_r[r + 1] - schedstart_r[r])
                    kend = min((k + 1) * CH, N)
                    oht = opool.tile([128, max_sched * P], BF16, tag="oh")
                    nc.scalar.dma_start(
                        out=oht[:, : nsched * P],
                        in_=ohp[:, int(schedstart_r[r]) * P :
                                int(schedstart_r[r + 1]) * P])
                    gbuf = gpool.tile([P, max_blk, P], BF16, tag="gath")
                    a16 = int(blkstart_r[r]) * 8
                    nc.gpsimd.dma_gather(
                        gbuf[:, :nblk, :], T_full[k * CH : kend, :],
                        gidx_t[:, a16 : a16 + s_gk // 16], s_gk, s_gk, HID,
                        single_packet=False,
                    )
                    for (j, t, ohc, stop) in sched[r]:
                        lc = ohc - int(schedstart_r[r])
                        nc.tensor.matmul(
                            out=agg_ap(t),
                            lhsT=gbuf[:, j, :],
                            rhs=oht[:, lc * P : (lc + 1) * P],
                            start=False,
                            stop=stop,
                            skip_group_check=True,
                        )

                # ---- epilogue per tile ----
                for t in tiles:
                    rows = min(P, NSH - t * P)
                    a1 = agg_ap(t)
                    e_t = zpool.tile([P, P], F32, tag="e")
                    nc.scalar.activation(e_t[:], a1, AF.Exp, bias=bias_t[:])
                    p_t = zpool.tile([P, P], BF16, tag="p")
                    nc.scalar.activation(p_t[:], e_t[:], AF.Relu,
                                         bias=1.0, scale=-1.0)
                    r_t = zpool.tile([P, P], BF16, tag="r")
                    nc.scalar.activation(r_t[:], a1, AF.Relu, bias=bias_t[:])
                    if layer == 1:
                        ps2 = wpsum.tile([P, HID], F32, tag="wps", space="PSUM")
                        nc.tensor.matmul(out=ps2[:], lhsT=r_t[:], rhs=w1h_t[:],
                                         start=True, stop=False)
                        nc.tensor.matmul(out=ps2[:], lhsT=r_t[:], rhs=w1l_t[:],
                                         start=False, stop=False)
                        nc.tensor.matmul(out=ps2[:], lhsT=p_t[:], rhs=nw1h_t[:],
                                         start=False, stop=False)
                        nc.tensor.matmul(out=ps2[:], lhsT=p_t[:], rhs=nw1l_t[:],
                                         start=False, stop=True)
                        t2b = out_tiles[t]
                        nc.scalar.activation(t2b[:], ps2[:], AF.Copy)
                        nc.sync.dma_start(out=t2_shard[t * P : t * P + rows, :],
                                          in_=t2b[:rows, :])
                    else:
                        psw = wpsum.tile([P, HID], F32, tag="wps", space="PSUM")
                        ps3 = psw[:, :C_OUT]
                        nc.tensor.matmul(out=ps3, lhsT=r_t[:], rhs=wlh_t[:],
                                         start=True, stop=False)
                        nc.tensor.matmul(out=ps3, lhsT=r_t[:], rhs=wll_t[:],
                                         start=False, stop=False)
                        nc.tensor.matmul(out=ps3, lhsT=p_t[:], rhs=nwlh_t[:],
                                         start=False, stop=False)
                        nc.tensor.matmul(out=ps3, lhsT=p_t[:], rhs=nwll_t[:],
                                         start=False, stop=False)
                        nc.tensor.matmul(out=ps3, lhsT=ones_t[:], rhs=blr_t[:],
                                         start=False, stop=True)
                        lg = zpool.tile([P, C_OUT], F32, tag="lg")
                        nc.scalar.activation(lg[:], ps3, AF.Copy)
                        mx = zpool.tile([P, 1], F32, tag="mx")
                        nc.vector.tensor_reduce(out=mx[:], in_=lg[:],
                                                axis=mybir.AxisListType.X,
                                                op=mybir.AluOpType.max)
                        nmx = zpool.tile([P, 1], F32, tag="nmx")
                        nc.scalar.activation(nmx[:], mx[:], AF.Copy, scale=-1.0)
                        exd = zpool.tile([P, C_OUT], BF16, tag="exd")
                        sm = zpool.tile([P, 1], F32, tag="sm")
                        nc.scalar.activation(exd[:], lg[:], AF.Exp,
                                             bias=nmx[:], accum_out=sm[:])
                        lnsm = zpool.tile([P, 1], F32, tag="lnsm")
                        nc.scalar.activation(lnsm[:], sm[:], AF.Ln)
                        nlsn = zpool.tile([P, 1], F32, tag="nlsn")
                        nc.scalar.activation(nlsn[:], lnsm[:], AF.Identity,
                                             bias=nmx[:], scale=-1.0)
                        res = zpool.tile([P, C_OUT], F32, tag="res")
                        nc.scalar.activation(res[:], lg[:], AF.Identity,
                                             bias=nlsn[:])
                        nc.sync.dma_start(out=out_ext[t * P : t * P + rows, :],
                                          in_=res[:rows, :])

        agg_layer(T1_full, t1_tiles, t2_tiles, 1)
        nc.gpsimd.collective_compute(
            "AllGather", mybir.AluOpType.bypass,
            replica_groups=[list(range(NCORES))],
            ins=[t2_shard[:].opt()], outs=[T2_full[:].opt()],
        )
        agg_layer(T2_full, t2_tiles, None, 2)

    nc.finalize()
    return nc


_CACHE = {}


def kernel(**inputs):
    in_maps, meta = _prep_host(
        inputs["x"], inputs["edge_index"], inputs["W0"], inputs["b0"],
        inputs["W1"], inputs["b1"], inputs["Wl"], inputs["bl"])
    key = (meta["tot_blocks"], meta["tot_sched"],
           meta["nblk_r"].tobytes(),
           str(meta["sched"]).__hash__())
    if key not in _CACHE:
        _CACHE[key] = _build_program(meta)
    nc = _CACHE[key]
    trace = bool(int(__import__("os").environ.get("KERNEL_TRACE", "0")))
    res = run_bass_kernel_spmd(nc, in_maps, list(range(NCORES)), trace=trace)
    kernel.last_results = res
    out = np.concatenate([res.results[c]["out"] for c in range(NCORES)], axis=0)
    return out.astype(np.float32)


# revision 3
# speedup vs baseline: 1.5352x; 1.4800x over previous
"""2-layer GCN + classifier on 8 Trainium2 NeuronCores — v4.

v2 + gather-padding trim and pipeline gap removal:
- Edge blocks packed per (tile-group, chunk) run instead of per (tile, chunk):
  ~2% padding instead of ~19%. Blocks may straddle dst-tile boundaries; the
  matmul schedule is the UNION over cores of (block, tile) pairs, with
  per-core one-hot columns zeroing non-local lanes.
- Whole-layer gather index stream loaded in ONE DMA up front (gathers never
  wait on index loads).
- T-shard tiles pinned in SBUF from the moment they are produced, so the
  self-loop diagonal matmuls need no reloads.
- One-hot slabs + sdiag slabs loaded on the Activation HWDGE queue, keeping
  the SP queue short.
"""
import sys

sys.path.insert(0, "/opt/trn_rl_repo")

import numpy as np
import ml_dtypes

import concourse.bacc as bacc
import concourse.tile as tile
from concourse import mybir
from concourse.bass_utils import run_bass_kernel_spmd

N = 100000
E = 1600000
F_IN = 128
HID = 128
C_OUT = 40
NCORES = 8
NSH = N // NCORES          # 12500
P = 128
NT = (NSH + P - 1) // P    # 98
NSH_PAD = NT * P           # 12544
CH = 32768
NCHUNK = (N + CH - 1) // CH  # 4
TG = 12
NTG = (NT + TG - 1) // TG  # 9
NRUN = NTG * NCHUNK

F32 = mybir.dt.float32
BF16 = mybir.dt.bfloat16
I16 = mybir.dt.int16


def _split_hi_lo(w):
    hi = w.astype(ml_dtypes.bfloat16)
    lo = (w - hi.astype(np.float32)).astype(ml_dtypes.bfloat16)
    return hi, lo


def _prep_host(x, edge_index, W0, b0, W1, b1, Wl, bl):
    src = np.asarray(edge_index[0]).astype(np.int64)
    dst = np.asarray(edge_index[1]).astype(np.int64)
    loop = np.arange(N, dtype=np.int64)
    deg = np.bincount(np.concatenate([dst, loop]), minlength=N).astype(np.float64)
    dinv = 1.0 / np.sqrt(deg)
    wnorm = (dinv[src] * dinv[dst]).astype(np.float32)

    sdiag_all = np.zeros((NCORES, 128, NSH_PAD), dtype=ml_dtypes.bfloat16)
    for c in range(NCORES):
        nodes = np.arange(NSH, dtype=np.int64)
        sdiag_all[c][nodes % P, nodes] = (1.0 / deg[c * NSH + nodes]).astype(
            np.float32
        )

    core_of = dst // NSH
    per_core = []
    len_rc = np.zeros((NCORES, NRUN), dtype=np.int64)
    for c in range(NCORES):
        sel = core_of == c
        es = src[sel]
        ed = dst[sel] - c * NSH
        w = wnorm[sel]
        t_id = ed // P
        k_id = es // CH
        g_id = t_id // TG
        run = g_id * NCHUNK + k_id
        key = run * NT + t_id
        order = np.argsort(key, kind="stable")
        es, ed, w, run, t_id = (es[order], ed[order], w[order],
                                run[order], t_id[order])
        len_rc[c] = np.bincount(run, minlength=NRUN)
        per_core.append((es, ed % P, w, run, t_id, k_id[order]))

    nblk_r = np.ceil(len_rc.max(axis=0) / P).astype(np.int64)   # [NRUN]
    blkstart_r = np.zeros(NRUN + 1, dtype=np.int64)
    np.cumsum(nblk_r, out=blkstart_r[1:])
    tot_blocks = int(blkstart_r[-1])
    tot_slots = tot_blocks * P

    # per-core (run, block, tile) triples -> union schedule
    MAXJ = int(nblk_r.max()) + 1
    codes_per_core = []
    pos_per_core = []
    for c in range(NCORES):
        es, slot, w, run, t_id, k_s = per_core[c]
        runstart = np.zeros(NRUN + 1, dtype=np.int64)
        np.cumsum(len_rc[c], out=runstart[1:])
        pos = np.arange(len(es), dtype=np.int64) - runstart[run]
        j = pos // P
        code = (run * MAXJ + j) * NT + t_id
        codes_per_core.append(code)
        pos_per_core.append(pos)

    union = np.unique(np.concatenate(codes_per_core))   # sorted (run, j, t)
    tot_sched = len(union)
    u_run = union // (MAXJ * NT)
    u_j = (union // NT) % MAXJ
    u_t = union % NT
    # sched entries for run r occupy ohcols [schedstart_r[r], schedstart_r[r+1])
    schedstart_r = np.searchsorted(u_run, np.arange(NRUN + 1))

    # stop flags: per group g, last sched entry (over k asc, entry asc) per tile
    stop_flag = np.zeros(tot_sched, dtype=bool)
    t_has = [set() for _ in range(NTG)]
    for g in range(NTG):
        last_for_t = {}
        for k in range(NCHUNK):
            r = g * NCHUNK + k
            for s in range(schedstart_r[r], schedstart_r[r + 1]):
                last_for_t[int(u_t[s])] = s
                t_has[g].add(int(u_t[s]))
        for t, s in last_for_t.items():
            stop_flag[s] = True

    sched = []   # per run: list of (j, t, ohcol, stop)
    for r in range(NRUN):
        ent = [(int(u_j[s]), int(u_t[s]), int(s), bool(stop_flag[s]))
               for s in range(schedstart_r[r], schedstart_r[r + 1])]
        sched.append(ent)

    gidx_all = np.zeros((NCORES, 128, tot_slots // 16), dtype=np.int16)
    oh_all = np.zeros((NCORES, 128, tot_sched * P), dtype=ml_dtypes.bfloat16)
    for c in range(NCORES):
        es, slot, w, run, t_id, k_s = per_core[c]
        pos = pos_per_core[c]
        idxflat = np.zeros(tot_slots, dtype=np.int16)
        idxflat[blkstart_r[run] * P + pos] = (es - k_s * CH).astype(np.int16)
        gidx_all[c] = np.tile(idxflat.reshape(-1, 16).T, (8, 1))
        ohcol = np.searchsorted(union, codes_per_core[c])
        oh_all[c][pos % P, ohcol * P + slot] = w

    xT_hi = np.zeros((NCORES, 128, NSH_PAD), dtype=ml_dtypes.bfloat16)
    xT_lo = np.zeros((NCORES, 128, NSH_PAD), dtype=ml_dtypes.bfloat16)
    for c in range(NCORES):
        xs = np.asarray(x[c * NSH : (c + 1) * NSH]).astype(np.float32).T
        hi, lo = _split_hi_lo(xs)
        xT_hi[c, :, :NSH] = hi
        xT_lo[c, :, :NSH] = lo
    xf = np.asarray(x).astype(np.float32).T          # [128, N] replicated
    xTf_hi, xTf_lo = _split_hi_lo(xf)

    W0h, W0l = _split_hi_lo(np.asarray(W0, dtype=np.float32))
    W1h, W1l = _split_hi_lo(np.asarray(W1, dtype=np.float32))
    Wlh, Wll = _split_hi_lo(np.asarray(Wl, dtype=np.float32))
    b0c = np.asarray(b0, dtype=np.float32).reshape(128, 1)
    b1c = np.asarray(b1, dtype=np.float32).reshape(128, 1)
    blrow = np.asarray(bl, dtype=np.float32).reshape(1, C_OUT).astype(
        ml_dtypes.bfloat16)

    in_maps = []
    for c in range(NCORES):
        in_maps.append(
            {
                "xT_hi": xT_hi[c], "xT_lo": xT_lo[c],
                "xTf_hi": xTf_hi, "xTf_lo": xTf_lo,
                "gidx": gidx_all[c], "oh": oh_all[c], "sdiag": sdiag_all[c],
                "W0h": W0h, "W0l": W0l,
                "W1h": W1h, "W1l": W1l,
                "nW1h": -W1h, "nW1l": -W1l,
                "Wlh": Wlh, "Wll": Wll,
                "nWlh": -Wlh, "nWll": -Wll,
                "b0c": b0c, "b1c": b1c, "blrow": blrow,
            }
        )
    meta = dict(sched=sched, nblk_r=nblk_r, blkstart_r=blkstart_r,
                schedstart_r=schedstart_r, t_has=t_has,
                tot_blocks=tot_blocks, tot_slots=tot_slots,
                tot_sched=tot_sched)
    return in_maps, meta


def _build_program(meta):
    sched = meta["sched"]
    nblk_r = meta["nblk_r"]
    blkstart_r = meta["blkstart_r"]
    schedstart_r = meta["schedstart_r"]
    t_has = meta["t_has"]
    tot_blocks = meta["tot_blocks"]
    tot_slots = meta["tot_slots"]
    tot_sched = meta["tot_sched"]
    max_blk = int(nblk_r.max())
    max_sched = int(max(schedstart_r[r + 1] - schedstart_r[r]
                        for r in range(NRUN)))

    nc = bacc.Bacc(num_devices=NCORES, num_swdge_queues=2)
    xT_hi = nc.declare_dram_parameter("xT_hi", [128, NSH_PAD], BF16, isOutput=False)
    xT_lo = nc.declare_dram_parameter("xT_lo", [128, NSH_PAD], BF16, isOutput=False)
    xTf_hi = nc.declare_dram_parameter("xTf_hi", [128, N], BF16, isOutput=False)
    xTf_lo = nc.declare_dram_parameter("xTf_lo", [128, N], BF16, isOutput=False)
    gidx = nc.declare_dram_parameter("gidx", [128, tot_slots // 16], I16,
                                     isOutput=False)
    ohp = nc.declare_dram_parameter("oh", [128, tot_sched * P], BF16,
                                    isOutput=False)
    sdiag = nc.declare_dram_parameter("sdiag", [128, NSH_PAD], BF16,
                                      isOutput=False)
    W0h = nc.declare_dram_parameter("W0h", [128, HID], BF16, isOutput=False)
    W0l = nc.declare_dram_parameter("W0l", [128, HID], BF16, isOutput=False)
    W1h = nc.declare_dram_parameter("W1h", [128, HID], BF16, isOutput=False)
    W1l = nc.declare_dram_parameter("W1l", [128, HID], BF16, isOutput=False)
    nW1h = nc.declare_dram_parameter("nW1h", [128, HID], BF16, isOutput=False)
    nW1l = nc.declare_dram_parameter("nW1l", [128, HID], BF16, isOutput=False)
    Wlh = nc.declare_dram_parameter("Wlh", [128, C_OUT], BF16, isOutput=False)
    Wll = nc.declare_dram_parameter("Wll", [128, C_OUT], BF16, isOutput=False)
    nWlh = nc.declare_dram_parameter("nWlh", [128, C_OUT], BF16, isOutput=False)
    nWll = nc.declare_dram_parameter("nWll", [128, C_OUT], BF16, isOutput=False)
    b0c = nc.declare_dram_parameter("b0c", [128, 1], F32, isOutput=False)
    b1c = nc.declare_dram_parameter("b1c", [128, 1], F32, isOutput=False)
    blrow = nc.declare_dram_parameter("blrow", [1, C_OUT], BF16, isOutput=False)
    out_ext = nc.declare_dram_parameter("out", [NSH, C_OUT], F32, isOutput=True)

    t2_shard = nc.dram_tensor("t2_shard", [NSH, HID], BF16)
    T1_ck = [nc.dram_tensor(f"T1_c{k}", [min((k + 1) * CH, N) - k * CH, HID],
                            BF16) for k in range(NCHUNK)]
    T2_full = nc.dram_tensor("T2_full", [N, HID], BF16, addr_space="Shared")

    AF = mybir.ActivationFunctionType

    from contextlib import ExitStack
    with tile.TileContext(nc) as tc, ExitStack() as es:
        cpool = es.enter_context(tc.tile_pool(name="const", bufs=1))
        tpool = es.enter_context(tc.tile_pool(name="tsh", bufs=1))
        xpool = es.enter_context(tc.tile_pool(name="xp", bufs=3))
        gpool = es.enter_context(tc.tile_pool(name="gp", bufs=2))
        opool = es.enter_context(tc.tile_pool(name="ohp", bufs=2))
        dpool = es.enter_context(tc.tile_pool(name="dg", bufs=2))
        zpool = es.enter_context(tc.tile_pool(name="zp", bufs=4))
        apsum = es.enter_context(tc.tile_pool(name="apsum", bufs=2, space="PSUM"))
        wpsum = es.enter_context(tc.tile_pool(name="wpsum", bufs=2, space="PSUM"))

        # ---- constants ----
        w0h_t = cpool.tile([128, HID], BF16, tag="w0h")
        w0l_t = cpool.tile([128, HID], BF16, tag="w0l")
        w1h_t = cpool.tile([128, HID], BF16, tag="w1h")
        w1l_t = cpool.tile([128, HID], BF16, tag="w1l")
        nw1h_t = cpool.tile([128, HID], BF16, tag="nw1h")
        nw1l_t = cpool.tile([128, HID], BF16, tag="nw1l")
        wlh_t = cpool.tile([128, C_OUT], BF16, tag="wlh")
        wll_t = cpool.tile([128, C_OUT], BF16, tag="wll")
        nwlh_t = cpool.tile([128, C_OUT], BF16, tag="nwlh")
        nwll_t = cpool.tile([128, C_OUT], BF16, tag="nwll")
        b0_t = cpool.tile([128, 1], F32, tag="b0")
        b1_t = cpool.tile([128, 1], F32, tag="b1")
        blr_t = cpool.tile([1, C_OUT], BF16, tag="blr")
        for tt, ext in [(w0h_t, W0h), (w0l_t, W0l), (w1h_t, W1h), (w1l_t, W1l),
                        (nw1h_t, nW1h), (nw1l_t, nW1l),
                        (wlh_t, Wlh), (wll_t, Wll), (nwlh_t, nWlh),
                        (nwll_t, nWll), (b0_t, b0c), (b1_t, b1c)]:
            nc.sync.dma_start(out=tt[:], in_=ext[:, :])
        nc.sync.dma_start(out=blr_t[:], in_=blrow[:, :])
        ones_t = cpool.tile([1, P], BF16, tag="ones")
        nc.vector.memset(ones_t[:], 1.0)

        # whole-program gather index stream (shared by both layers)
        gidx_t = cpool.tile([128, tot_slots // 16], I16, tag="gidx")
        nc.sync.dma_start(out=gidx_t[:], in_=gidx[:, :])

        # pinned T-shard tiles
        t1_tiles = [tpool.tile([P, HID], BF16, tag=f"t1_{t}", name=f"t1_{t}")
                    for t in range(NT)]
        t2_tiles = [tpool.tile([P, HID], BF16, tag=f"t2_{t}", name=f"t2_{t}")
                    for t in range(NT)]

        # ---- phase 1a: own-shard T1 tiles (pinned, for self-loop diag) ----
        for t in range(NT):
            xh = xpool.tile([128, P], BF16, tag="xh")
            xl = xpool.tile([128, P], BF16, tag="xl")
            nc.sync.dma_start(out=xh[:], in_=xT_hi[:, t * P : (t + 1) * P])
            nc.sync.dma_start(out=xl[:], in_=xT_lo[:, t * P : (t + 1) * P])
            ps = wpsum.tile([P, HID], F32, tag="wps", space="PSUM")
            nc.tensor.matmul(out=ps[:], lhsT=xh[:], rhs=w0h_t[:], start=True, stop=False)
            nc.tensor.matmul(out=ps[:], lhsT=xh[:], rhs=w0l_t[:], start=False, stop=False)
            nc.tensor.matmul(out=ps[:], lhsT=xl[:], rhs=w0h_t[:], start=False, stop=True)
            tb = t1_tiles[t]
            nc.scalar.activation(tb[:], ps[:], AF.Copy)

        # ---- phase 1b: full T1 computed locally (replaces AllGather #1) ----
        NGT = (N + P - 1) // P
        for gt in range(NGT):
            rows = min(P, N - gt * P)
            k = (gt * P) // CH
            xh = xpool.tile([128, P], BF16, tag="xh")
            xl = xpool.tile([128, P], BF16, tag="xl")
            nc.sync.dma_start(out=xh[:, :rows], in_=xTf_hi[:, gt * P : gt * P + rows])
            nc.sync.dma_start(out=xl[:, :rows], in_=xTf_lo[:, gt * P : gt * P + rows])
            ps = wpsum.tile([P, HID], F32, tag="wps", space="PSUM")
            nc.tensor.matmul(out=ps[:rows, :], lhsT=xh[:, :rows], rhs=w0h_t[:],
                             start=True, stop=False)
            nc.tensor.matmul(out=ps[:rows, :], lhsT=xh[:, :rows], rhs=w0l_t[:],
                             start=False, stop=False)
            nc.tensor.matmul(out=ps[:rows, :], lhsT=xl[:, :rows], rhs=w0h_t[:],
                             start=False, stop=True)
            fb = xpool.tile([P, HID], BF16, tag="fb")
            nc.scalar.activation(fb[:rows, :], ps[:rows, :], AF.Copy)
            a = gt * P - k * CH
            nc.sync.dma_start(out=T1_ck[k][a : a + rows, :], in_=fb[:rows, :])

        def agg_layer(gsrc, t_tiles, out_tiles, layer):
            bias_t = b0_t if layer == 1 else b1_t
            for g in range(NTG):
                tiles = list(range(g * TG, min((g + 1) * TG, NT)))
                nbank = (len(tiles) + 3) // 4
                banks = [apsum.tile([P, 512], F32, tag=f"agg{i}", space="PSUM",
                                    name=f"aggbank{i}")
                         for i in range(nbank)]

                def agg_ap(ti):
                    i = tiles.index(ti)
                    return banks[i // 4][:, (i % 4) * P : (i % 4 + 1) * P]

                # sdiag slab for this group (Act HWDGE queue)
                sds = dpool.tile([128, TG * P], BF16, tag="sds")
                nc.scalar.dma_start(
                    out=sds[:, : len(tiles) * P],
                    in_=sdiag[:, g * TG * P : g * TG * P + len(tiles) * P])

                # self-loop diagonal opens each tile's PSUM group
                for i, t in enumerate(tiles):
                    rows = min(P, NSH - t * P)
                    nc.tensor.matmul(out=agg_ap(t), lhsT=t_tiles[t][:rows, :],
                                     rhs=sds[:rows, i * P : (i + 1) * P],
                                     start=True, stop=(t not in t_has[g]),
                                     skip_group_check=True)

                for k in range(NCHUNK):
                    r = g * NCHUNK + k
                    nblk = int(nblk_r[r])
                    if nblk == 0:
                        continue
                    s_gk = nblk * P
                    nsched = int(schedstart_r[r + 1] - schedstart_r[r])
                    kend = min((k + 1) * CH, N)
                    oht = opool.tile([128, max_sched * P], BF16, tag="oh")
                    nc.scalar.dma_start(
                        out=oht[:, : nsched * P],
                        in_=ohp[:, int(schedstart_r[r]) * P :
                                int(schedstart_r[r + 1]) * P])
                    gbuf = gpool.tile([P, max_blk, P], BF16, tag="gath")
                    a16 = int(blkstart_r[r]) * 8
                    nc.gpsimd.dma_gather(
                        gbuf[:, :nblk, :], gsrc(k, kend),
                        gidx_t[:, a16 : a16 + s_gk // 16], s_gk, s_gk, HID,
                        single_packet=False, queue_num=(k % 2),
                    )
                    for (j, t, ohc, stop) in sched[r]:
                        lc = ohc - int(schedstart_r[r])
                        nc.tensor.matmul(
                            out=agg_ap(t),
                            lhsT=gbuf[:, j, :],
                            rhs=oht[:, lc * P : (lc + 1) * P],
                            start=False,
                            stop=stop,
                            skip_group_check=True,
                        )

                # ---- epilogue per tile ----
                for t in tiles:
                    rows = min(P, NSH - t * P)
                    a1 = agg_ap(t)
                    e_t = zpool.tile([P, P], F32, tag="e")
                    nc.scalar.activation(e_t[:], a1, AF.Exp, bias=bias_t[:])
                    p_t = zpool.tile([P, P], BF16, tag="p")
                    nc.scalar.activation(p_t[:], e_t[:], AF.Relu,
                                         bias=1.0, scale=-1.0)
                    r_t = zpool.tile([P, P], BF16, tag="r")
                    nc.scalar.activation(r_t[:], a1, AF.Relu, bias=bias_t[:])
                    if layer == 1:
                        ps2 = wpsum.tile([P, HID], F32, tag="wps", space="PSUM")
                        nc.tensor.matmul(out=ps2[:], lhsT=r_t[:], rhs=w1h_t[:],
                                         start=True, stop=False)
                        nc.tensor.matmul(out=ps2[:], lhsT=r_t[:], rhs=w1l_t[:],
                                         start=False, stop=False)
                        nc.tensor.matmul(out=ps2[:], lhsT=p_t[:], rhs=nw1h_t[:],
                                         start=False, stop=False)
                        nc.tensor.matmul(out=ps2[:], lhsT=p_t[:], rhs=nw1l_t[:],
                                         start=False, stop=True)
                        t2b = out_tiles[t]
                        nc.scalar.activation(t2b[:], ps2[:], AF.Copy)
                        nc.sync.dma_start(out=t2_shard[t * P : t * P + rows, :],
                                          in_=t2b[:rows, :])
                    else:
                        psw = wpsum.tile([P, HID], F32, tag="wps", space="PSUM")
                        ps3 = psw[:, :C_OUT]
                        nc.tensor.matmul(out=ps3, lhsT=r_t[:], rhs=wlh_t[:],
                                         start=True, stop=False)
                        nc.tensor.matmul(out=ps3, lhsT=r_t[:], rhs=wll_t[:],
                                         start=False, stop=False)
                        nc.tensor.matmul(out=ps3, lhsT=p_t[:], rhs=nwlh_t[:],
                                         start=False, stop=False)
                        nc.tensor.matmul(out=ps3, lhsT=p_t[:], rhs=nwll_t[:],
                                         start=False, stop=False)
                        nc.tensor.matmul(out=ps3, lhsT=ones_t[:], rhs=blr_t[:],
                                         start=False, stop=True)
                        lg = zpool.tile([P, C_OUT], F32, tag="lg")
                        nc.scalar.activation(lg[:], ps3, AF.Copy)
                        mx = zpool.tile([P, 1], F32, tag="mx")
                        nc.vector.tensor_reduce(out=mx[:], in_=lg[:],
                                                axis=mybir.AxisListType.X,
                                                op=mybir.AluOpType.max)
                        nmx = zpool.tile([P, 1], F32, tag="nmx")
                        nc.scalar.activation(nmx[:], mx[:], AF.Copy, scale=-1.0)
                        exd = zpool.tile([P, C_OUT], BF16, tag="exd")
                        sm = zpool.tile([P, 1], F32, tag="sm")
                        nc.scalar.activation(exd[:], lg[:], AF.Exp,
                                             bias=nmx[:], accum_out=sm[:])
                        lnsm = zpool.tile([P, 1], F32, tag="lnsm")
                        nc.scalar.activation(lnsm[:], sm[:], AF.Ln)
                        nlsn = zpool.tile([P, 1], F32, tag="nlsn")
                        nc.scalar.activation(nlsn[:], lnsm[:], AF.Identity,
                                             bias=nmx[:], scale=-1.0)
                        res = zpool.tile([P, C_OUT], F32, tag="res")
                        nc.scalar.activation(res[:], lg[:], AF.Identity,
                                             bias=nlsn[:])
                        nc.sync.dma_start(out=out_ext[t * P : t * P + rows, :],
                                          in_=res[:rows, :])

        agg_layer(lambda k, kend: T1_ck[k][:, :], t1_tiles, t2_tiles, 1)
        nc.gpsimd.collective_compute(
            "AllGather", mybir.AluOpType.bypass,
            replica_groups=[list(range(NCORES))],
            ins=[t2_shard[:].opt()], outs=[T2_full[:].opt()],
        )
        agg_layer(lambda k, kend: T2_full[k * CH : kend, :], t2_tiles, None, 2)

    nc.finalize()
    return nc


_CACHE = {}


def kernel(**inputs):
    in_maps, meta = _prep_host(
        inputs["x"], inputs["edge_index"], inputs["W0"], inputs["b0"],
        inputs["W1"], inputs["b1"], inputs["Wl"], inputs["bl"])
    key = (meta["tot_blocks"], meta["tot_sched"],
           meta["nblk_r"].tobytes(),
           str(meta["sched"]).__hash__())
    if key not in _CACHE:
        _CACHE[key] = _build_program(meta)
    nc = _CACHE[key]
    trace = bool(int(__import__("os").environ.get("KERNEL_TRACE", "0")))
    res = run_bass_kernel_spmd(nc, in_maps, list(range(NCORES)), trace=trace)
    kernel.last_results = res
    out = np.concatenate([res.results[c]["out"] for c in range(NCORES)], axis=0)
    return out.astype(np.float32)


# revision 4
# speedup vs baseline: 2.0224x; 1.3173x over previous
"""2-layer GCN + classifier on 8 Trainium2 NeuronCores — v5.

v2 + gather-padding trim and pipeline gap removal:
- Edge blocks packed per (tile-group, chunk) run instead of per (tile, chunk):
  ~2% padding instead of ~19%. Blocks may straddle dst-tile boundaries; the
  matmul schedule is the UNION over cores of (block, tile) pairs, with
  per-core one-hot columns zeroing non-local lanes.
- Whole-layer gather index stream loaded in ONE DMA up front (gathers never
  wait on index loads).
- T-shard tiles pinned in SBUF from the moment they are produced, so the
  self-loop diagonal matmuls need no reloads.
- One-hot slabs + sdiag slabs loaded on the Activation HWDGE queue, keeping
  the SP queue short.
"""
import sys

sys.path.insert(0, "/opt/trn_rl_repo")

import numpy as np
import ml_dtypes

import concourse.bacc as bacc
import concourse.tile as tile
from concourse import mybir
from concourse.bass_utils import run_bass_kernel_spmd

N = 100000
E = 1600000
F_IN = 128
HID = 128
C_OUT = 40
NCORES = 8
NSH = N // NCORES          # 12500
P = 128
NT = (NSH + P - 1) // P    # 98
NSH_PAD = NT * P           # 12544
CH = 32768
NCHUNK = (N + CH - 1) // CH  # 4
TG = 12
NTG = (NT + TG - 1) // TG  # 9
NRUN = NTG * NCHUNK

F32 = mybir.dt.float32
BF16 = mybir.dt.bfloat16
I16 = mybir.dt.int16


def _split_hi_lo(w):
    hi = w.astype(ml_dtypes.bfloat16)
    lo = (w - hi.astype(np.float32)).astype(ml_dtypes.bfloat16)
    return hi, lo


def _prep_host(x, edge_index, W0, b0, W1, b1, Wl, bl):
    src = np.asarray(edge_index[0]).astype(np.int64)
    dst = np.asarray(edge_index[1]).astype(np.int64)
    loop = np.arange(N, dtype=np.int64)
    deg = np.bincount(np.concatenate([dst, loop]), minlength=N).astype(np.float64)
    dinv = 1.0 / np.sqrt(deg)
    wnorm = (dinv[src] * dinv[dst]).astype(np.float32)

    sdiag_all = np.zeros((NCORES, 128, NSH_PAD), dtype=ml_dtypes.bfloat16)
    for c in range(NCORES):
        nodes = np.arange(NSH, dtype=np.int64)
        sdiag_all[c][nodes % P, nodes] = (1.0 / deg[c * NSH + nodes]).astype(
            np.float32
        )

    core_of = dst // NSH
    per_core = []
    len_rc = np.zeros((NCORES, NRUN), dtype=np.int64)
    for c in range(NCORES):
        sel = core_of == c
        es = src[sel]
        ed = dst[sel] - c * NSH
        w = wnorm[sel]
        t_id = ed // P
        k_id = es // CH
        g_id = t_id // TG
        run = g_id * NCHUNK + k_id
        key = run * NT + t_id
        order = np.argsort(key, kind="stable")
        es, ed, w, run, t_id = (es[order], ed[order], w[order],
                                run[order], t_id[order])
        len_rc[c] = np.bincount(run, minlength=NRUN)
        per_core.append((es, ed % P, w, run, t_id, k_id[order]))

    nblk_r = np.ceil(len_rc.max(axis=0) / P).astype(np.int64)   # [NRUN]
    blkstart_r = np.zeros(NRUN + 1, dtype=np.int64)
    np.cumsum(nblk_r, out=blkstart_r[1:])
    tot_blocks = int(blkstart_r[-1])
    tot_slots = tot_blocks * P

    # per-core (run, block, tile) triples -> union schedule
    MAXJ = int(nblk_r.max()) + 1
    codes_per_core = []
    pos_per_core = []
    for c in range(NCORES):
        es, slot, w, run, t_id, k_s = per_core[c]
        runstart = np.zeros(NRUN + 1, dtype=np.int64)
        np.cumsum(len_rc[c], out=runstart[1:])
        pos = np.arange(len(es), dtype=np.int64) - runstart[run]
        j = pos // P
        code = (run * MAXJ + j) * NT + t_id
        codes_per_core.append(code)
        pos_per_core.append(pos)

    union = np.unique(np.concatenate(codes_per_core))   # sorted (run, j, t)
    tot_sched = len(union)
    u_run = union // (MAXJ * NT)
    u_j = (union // NT) % MAXJ
    u_t = union % NT
    # sched entries for run r occupy ohcols [schedstart_r[r], schedstart_r[r+1])
    schedstart_r = np.searchsorted(u_run, np.arange(NRUN + 1))

    # stop flags: per group g, last sched entry (over k asc, entry asc) per tile
    stop_flag = np.zeros(tot_sched, dtype=bool)
    t_has = [set() for _ in range(NTG)]
    for g in range(NTG):
        last_for_t = {}
        for k in range(NCHUNK):
            r = g * NCHUNK + k
            for s in range(schedstart_r[r], schedstart_r[r + 1]):
                last_for_t[int(u_t[s])] = s
                t_has[g].add(int(u_t[s]))
        for t, s in last_for_t.items():
            stop_flag[s] = True

    sched = []   # per run: list of (j, t, ohcol, stop)
    for r in range(NRUN):
        ent = [(int(u_j[s]), int(u_t[s]), int(s), bool(stop_flag[s]))
               for s in range(schedstart_r[r], schedstart_r[r + 1])]
        sched.append(ent)

    gidx_all = np.zeros((NCORES, 128, tot_slots // 16), dtype=np.int16)
    oh_all = np.zeros((NCORES, 128, tot_sched * P), dtype=ml_dtypes.bfloat16)
    for c in range(NCORES):
        es, slot, w, run, t_id, k_s = per_core[c]
        pos = pos_per_core[c]
        idxflat = np.zeros(tot_slots, dtype=np.int16)
        idxflat[blkstart_r[run] * P + pos] = (es - k_s * CH).astype(np.int16)
        gidx_all[c] = np.tile(idxflat.reshape(-1, 16).T, (8, 1))
        ohcol = np.searchsorted(union, codes_per_core[c])
        oh_all[c][pos % P, ohcol * P + slot] = w

    xT_hi = np.zeros((NCORES, 128, NSH_PAD), dtype=ml_dtypes.bfloat16)
    xT_lo = np.zeros((NCORES, 128, NSH_PAD), dtype=ml_dtypes.bfloat16)
    for c in range(NCORES):
        xs = np.asarray(x[c * NSH : (c + 1) * NSH]).astype(np.float32).T
        hi, lo = _split_hi_lo(xs)
        xT_hi[c, :, :NSH] = hi
        xT_lo[c, :, :NSH] = lo
    xf = np.asarray(x).astype(np.float32).T          # [128, N] replicated
    xTf_hi, xTf_lo = _split_hi_lo(xf)

    W0h, W0l = _split_hi_lo(np.asarray(W0, dtype=np.float32))
    W1h, W1l = _split_hi_lo(np.asarray(W1, dtype=np.float32))
    Wlh, Wll = _split_hi_lo(np.asarray(Wl, dtype=np.float32))
    b0c = np.asarray(b0, dtype=np.float32).reshape(128, 1)
    b1c = np.asarray(b1, dtype=np.float32).reshape(128, 1)
    blrow = np.asarray(bl, dtype=np.float32).reshape(1, C_OUT).astype(
        ml_dtypes.bfloat16)

    in_maps = []
    for c in range(NCORES):
        in_maps.append(
            {
                "xT_hi": xT_hi[c], "xT_lo": xT_lo[c],
                "xTf_hi": xTf_hi, "xTf_lo": xTf_lo,
                "gidx": gidx_all[c], "oh": oh_all[c], "sdiag": sdiag_all[c],
                "W0h": W0h, "W0l": W0l,
                "W1h": W1h, "W1l": W1l,
                "nW1h": -W1h, "nW1l": -W1l,
                "Wlh": Wlh, "Wll": Wll,
                "nWlh": -Wlh, "nWll": -Wll,
                "b0c": b0c, "b1c": b1c, "blrow": blrow,
            }
        )
    meta = dict(sched=sched, nblk_r=nblk_r, blkstart_r=blkstart_r,
                schedstart_r=schedstart_r, t_has=t_has,
                tot_blocks=tot_blocks, tot_slots=tot_slots,
                tot_sched=tot_sched)
    return in_maps, meta


def _build_program(meta):
    sched = meta["sched"]
    nblk_r = meta["nblk_r"]
    blkstart_r = meta["blkstart_r"]
    schedstart_r = meta["schedstart_r"]
    t_has = meta["t_has"]
    tot_blocks = meta["tot_blocks"]
    tot_slots = meta["tot_slots"]
    tot_sched = meta["tot_sched"]
    max_blk = int(nblk_r.max())
    max_sched = int(max(schedstart_r[r + 1] - schedstart_r[r]
                        for r in range(NRUN)))

    nc = bacc.Bacc(num_devices=NCORES, num_swdge_queues=4)
    xT_hi = nc.declare_dram_parameter("xT_hi", [128, NSH_PAD], BF16, isOutput=False)
    xT_lo = nc.declare_dram_parameter("xT_lo", [128, NSH_PAD], BF16, isOutput=False)
    xTf_hi = nc.declare_dram_parameter("xTf_hi", [128, N], BF16, isOutput=False)
    xTf_lo = nc.declare_dram_parameter("xTf_lo", [128, N], BF16, isOutput=False)
    gidx = nc.declare_dram_parameter("gidx", [128, tot_slots // 16], I16,
                                     isOutput=False)
    ohp = nc.declare_dram_parameter("oh", [128, tot_sched * P], BF16,
                                    isOutput=False)
    sdiag = nc.declare_dram_parameter("sdiag", [128, NSH_PAD], BF16,
                                      isOutput=False)
    W0h = nc.declare_dram_parameter("W0h", [128, HID], BF16, isOutput=False)
    W0l = nc.declare_dram_parameter("W0l", [128, HID], BF16, isOutput=False)
    W1h = nc.declare_dram_parameter("W1h", [128, HID], BF16, isOutput=False)
    W1l = nc.declare_dram_parameter("W1l", [128, HID], BF16, isOutput=False)
    nW1h = nc.declare_dram_parameter("nW1h", [128, HID], BF16, isOutput=False)
    nW1l = nc.declare_dram_parameter("nW1l", [128, HID], BF16, isOutput=False)
    Wlh = nc.declare_dram_parameter("Wlh", [128, C_OUT], BF16, isOutput=False)
    Wll = nc.declare_dram_parameter("Wll", [128, C_OUT], BF16, isOutput=False)
    nWlh = nc.declare_dram_parameter("nWlh", [128, C_OUT], BF16, isOutput=False)
    nWll = nc.declare_dram_parameter("nWll", [128, C_OUT], BF16, isOutput=False)
    b0c = nc.declare_dram_parameter("b0c", [128, 1], F32, isOutput=False)
    b1c = nc.declare_dram_parameter("b1c", [128, 1], F32, isOutput=False)
    blrow = nc.declare_dram_parameter("blrow", [1, C_OUT], BF16, isOutput=False)
    out_ext = nc.declare_dram_parameter("out", [NSH, C_OUT], F32, isOutput=True)

    t2_shard = nc.dram_tensor("t2_shard", [NSH, HID], BF16)
    T1_ck = [nc.dram_tensor(f"T1_c{k}", [min((k + 1) * CH, N) - k * CH, HID],
                            BF16) for k in range(NCHUNK)]
    T2_full = nc.dram_tensor("T2_full", [N, HID], BF16, addr_space="Shared")

    AF = mybir.ActivationFunctionType

    from contextlib import ExitStack
    with tile.TileContext(nc) as tc, ExitStack() as es:
        cpool = es.enter_context(tc.tile_pool(name="const", bufs=1))
        tpool = es.enter_context(tc.tile_pool(name="tsh", bufs=1))
        xpool = es.enter_context(tc.tile_pool(name="xp", bufs=3))
        gpool = es.enter_context(tc.tile_pool(name="gp", bufs=3))
        opool = es.enter_context(tc.tile_pool(name="ohp", bufs=2))
        dpool = es.enter_context(tc.tile_pool(name="dg", bufs=2))
        zpool = es.enter_context(tc.tile_pool(name="zp", bufs=4))
        apsum = es.enter_context(tc.tile_pool(name="apsum", bufs=2, space="PSUM"))
        wpsum = es.enter_context(tc.tile_pool(name="wpsum", bufs=2, space="PSUM"))

        # ---- constants ----
        w0h_t = cpool.tile([128, HID], BF16, tag="w0h")
        w0l_t = cpool.tile([128, HID], BF16, tag="w0l")
        w1h_t = cpool.tile([128, HID], BF16, tag="w1h")
        w1l_t = cpool.tile([128, HID], BF16, tag="w1l")
        nw1h_t = cpool.tile([128, HID], BF16, tag="nw1h")
        nw1l_t = cpool.tile([128, HID], BF16, tag="nw1l")
        wlh_t = cpool.tile([128, C_OUT], BF16, tag="wlh")
        wll_t = cpool.tile([128, C_OUT], BF16, tag="wll")
        nwlh_t = cpool.tile([128, C_OUT], BF16, tag="nwlh")
        nwll_t = cpool.tile([128, C_OUT], BF16, tag="nwll")
        b0_t = cpool.tile([128, 1], F32, tag="b0")
        b1_t = cpool.tile([128, 1], F32, tag="b1")
        blr_t = cpool.tile([1, C_OUT], BF16, tag="blr")
        for tt, ext in [(w0h_t, W0h), (w0l_t, W0l), (w1h_t, W1h), (w1l_t, W1l),
                        (nw1h_t, nW1h), (nw1l_t, nW1l),
                        (wlh_t, Wlh), (wll_t, Wll), (nwlh_t, nWlh),
                        (nwll_t, nWll), (b0_t, b0c), (b1_t, b1c)]:
            nc.sync.dma_start(out=tt[:], in_=ext[:, :])
        nc.sync.dma_start(out=blr_t[:], in_=blrow[:, :])
        ones_t = cpool.tile([1, P], BF16, tag="ones")
        nc.vector.memset(ones_t[:], 1.0)

        # whole-program gather index stream (shared by both layers)
        gidx_t = cpool.tile([128, tot_slots // 16], I16, tag="gidx")
        nc.sync.dma_start(out=gidx_t[:], in_=gidx[:, :])

        # pinned T-shard tiles
        t1_tiles = [tpool.tile([P, HID], BF16, tag=f"t1_{t}", name=f"t1_{t}")
                    for t in range(NT)]
        t2_tiles = [tpool.tile([P, HID], BF16, tag=f"t2_{t}", name=f"t2_{t}")
                    for t in range(NT)]

        # ---- phase 1a: own-shard T1 tiles (pinned, for self-loop diag) ----
        SL = 8
        for t0 in range(0, NT, SL):
            nt_s = min(SL, NT - t0)
            xh = xpool.tile([128, SL * P], BF16, tag="xh")
            xl = xpool.tile([128, SL * P], BF16, tag="xl")
            nc.sync.dma_start(out=xh[:, : nt_s * P],
                              in_=xT_hi[:, t0 * P : (t0 + nt_s) * P])
            nc.sync.dma_start(out=xl[:, : nt_s * P],
                              in_=xT_lo[:, t0 * P : (t0 + nt_s) * P])
            for i in range(nt_s):
                t = t0 + i
                ps = wpsum.tile([P, HID], F32, tag="wps", space="PSUM")
                nc.tensor.matmul(out=ps[:], lhsT=xh[:, i * P : (i + 1) * P],
                                 rhs=w0h_t[:], start=True, stop=False)
                nc.tensor.matmul(out=ps[:], lhsT=xh[:, i * P : (i + 1) * P],
                                 rhs=w0l_t[:], start=False, stop=False)
                nc.tensor.matmul(out=ps[:], lhsT=xl[:, i * P : (i + 1) * P],
                                 rhs=w0h_t[:], start=False, stop=True)
                tb = t1_tiles[t]
                nc.scalar.activation(tb[:], ps[:], AF.Copy)

        # ---- phase 1b: full T1 computed locally (replaces AllGather #1) ----
        NGT = (N + P - 1) // P
        for gt0 in range(0, NGT, SL):
            cols = min(SL * P, N - gt0 * P)
            nt_s = (cols + P - 1) // P
            xh = xpool.tile([128, SL * P], BF16, tag="xh")
            xl = xpool.tile([128, SL * P], BF16, tag="xl")
            nc.sync.dma_start(out=xh[:, :cols], in_=xTf_hi[:, gt0 * P : gt0 * P + cols])
            nc.sync.dma_start(out=xl[:, :cols], in_=xTf_lo[:, gt0 * P : gt0 * P + cols])
            for i in range(nt_s):
                gt = gt0 + i
                rows = min(P, N - gt * P)
                k = (gt * P) // CH
                ps = wpsum.tile([P, HID], F32, tag="wps", space="PSUM")
                nc.tensor.matmul(out=ps[:rows, :], lhsT=xh[:, i * P : i * P + rows],
                                 rhs=w0h_t[:], start=True, stop=False)
                nc.tensor.matmul(out=ps[:rows, :], lhsT=xh[:, i * P : i * P + rows],
                                 rhs=w0l_t[:], start=False, stop=False)
                nc.tensor.matmul(out=ps[:rows, :], lhsT=xl[:, i * P : i * P + rows],
                                 rhs=w0h_t[:], start=False, stop=True)
                fb = xpool.tile([P, HID], BF16, tag="fb")
                nc.scalar.activation(fb[:rows, :], ps[:rows, :], AF.Copy)
                a = gt * P - k * CH
                nc.scalar.dma_start(out=T1_ck[k][a : a + rows, :], in_=fb[:rows, :])

        def agg_layer(gsrc, t_tiles, out_tiles, layer):
            bias_t = b0_t if layer == 1 else b1_t
            for g in range(NTG):
                tiles = list(range(g * TG, min((g + 1) * TG, NT)))
                nbank = (len(tiles) + 3) // 4
                banks = [apsum.tile([P, 512], F32, tag=f"agg{i}", space="PSUM",
                                    name=f"aggbank{i}")
                         for i in range(nbank)]

                def agg_ap(ti):
                    i = tiles.index(ti)
                    return banks[i // 4][:, (i % 4) * P : (i % 4 + 1) * P]

                # sdiag slab for this group (Act HWDGE queue)
                sds = dpool.tile([128, TG * P], BF16, tag="sds")
                nc.scalar.dma_start(
                    out=sds[:, : len(tiles) * P],
                    in_=sdiag[:, g * TG * P : g * TG * P + len(tiles) * P])

                # self-loop diagonal opens each tile's PSUM group
                for i, t in enumerate(tiles):
                    rows = min(P, NSH - t * P)
                    nc.tensor.matmul(out=agg_ap(t), lhsT=t_tiles[t][:rows, :],
                                     rhs=sds[:rows, i * P : (i + 1) * P],
                                     start=True, stop=(t not in t_has[g]),
                                     skip_group_check=True)

                for k in range(NCHUNK):
                    r = g * NCHUNK + k
                    nblk = int(nblk_r[r])
                    if nblk == 0:
                        continue
                    s_gk = nblk * P
                    nsched = int(schedstart_r[r + 1] - schedstart_r[r])
                    kend = min((k + 1) * CH, N)
                    oht = opool.tile([128, max_sched * P], BF16, tag="oh")
                    nc.scalar.dma_start(
                        out=oht[:, : nsched * P],
                        in_=ohp[:, int(schedstart_r[r]) * P :
                                int(schedstart_r[r + 1]) * P])
                    gbuf = gpool.tile([P, max_blk, P], BF16, tag="gath")
                    a16 = int(blkstart_r[r]) * 8
                    nc.gpsimd.dma_gather(
                        gbuf[:, :nblk, :], gsrc(k, kend),
                        gidx_t[:, a16 : a16 + s_gk // 16], s_gk, s_gk, HID,
                        single_packet=False, queue_num=k,
                    )
                    for (j, t, ohc, stop) in sched[r]:
                        lc = ohc - int(schedstart_r[r])
                        nc.tensor.matmul(
                            out=agg_ap(t),
                            lhsT=gbuf[:, j, :],
                            rhs=oht[:, lc * P : (lc + 1) * P],
                            start=False,
                            stop=stop,
                            skip_group_check=True,
                        )

                # ---- epilogue per tile ----
                for t in tiles:
                    rows = min(P, NSH - t * P)
                    a1 = agg_ap(t)
                    e_t = zpool.tile([P, P], F32, tag="e")
                    nc.scalar.activation(e_t[:], a1, AF.Exp, bias=bias_t[:])
                    p_t = zpool.tile([P, P], BF16, tag="p")
                    nc.scalar.activation(p_t[:], e_t[:], AF.Relu,
                                         bias=1.0, scale=-1.0)
                    r_t = zpool.tile([P, P], BF16, tag="r")
                    nc.scalar.activation(r_t[:], a1, AF.Relu, bias=bias_t[:])
                    if layer == 1:
                        ps2 = wpsum.tile([P, HID], F32, tag="wps", space="PSUM")
                        nc.tensor.matmul(out=ps2[:], lhsT=r_t[:], rhs=w1h_t[:],
                                         start=True, stop=False)
                        nc.tensor.matmul(out=ps2[:], lhsT=r_t[:], rhs=w1l_t[:],
                                         start=False, stop=False)
                        nc.tensor.matmul(out=ps2[:], lhsT=p_t[:], rhs=nw1h_t[:],
                                         start=False, stop=False)
                        nc.tensor.matmul(out=ps2[:], lhsT=p_t[:], rhs=nw1l_t[:],
                                         start=False, stop=True)
                        t2b = out_tiles[t]
                        nc.scalar.activation(t2b[:], ps2[:], AF.Copy)
                        nc.sync.dma_start(out=t2_shard[t * P : t * P + rows, :],
                                          in_=t2b[:rows, :])
                    else:
                        psw = wpsum.tile([P, HID], F32, tag="wps", space="PSUM")
                        ps3 = psw[:, :C_OUT]
                        nc.tensor.matmul(out=ps3, lhsT=r_t[:], rhs=wlh_t[:],
                                         start=True, stop=False)
                        nc.tensor.matmul(out=ps3, lhsT=r_t[:], rhs=wll_t[:],
                                         start=False, stop=False)
                        nc.tensor.matmul(out=ps3, lhsT=p_t[:], rhs=nwlh_t[:],
                                         start=False, stop=False)
                        nc.tensor.matmul(out=ps3, lhsT=p_t[:], rhs=nwll_t[:],
                                         start=False, stop=False)
                        nc.tensor.matmul(out=ps3, lhsT=ones_t[:], rhs=blr_t[:],
                                         start=False, stop=True)
                        lg = zpool.tile([P, C_OUT], F32, tag="lg")
                        nc.scalar.activation(lg[:], ps3, AF.Copy)
                        mx = zpool.tile([P, 1], F32, tag="mx")
                        nc.vector.tensor_reduce(out=mx[:], in_=lg[:],
                                                axis=mybir.AxisListType.X,
                                                op=mybir.AluOpType.max)
                        nmx = zpool.tile([P, 1], F32, tag="nmx")
                        nc.scalar.activation(nmx[:], mx[:], AF.Copy, scale=-1.0)
                        exd = zpool.tile([P, C_OUT], BF16, tag="exd")
                        sm = zpool.tile([P, 1], F32, tag="sm")
                        nc.scalar.activation(exd[:], lg[:], AF.Exp,
                                             bias=nmx[:], accum_out=sm[:])
                        lnsm = zpool.tile([P, 1], F32, tag="lnsm")
                        nc.scalar.activation(lnsm[:], sm[:], AF.Ln)
                        nlsn = zpool.tile([P, 1], F32, tag="nlsn")
                        nc.scalar.activation(nlsn[:], lnsm[:], AF.Identity,
                                             bias=nmx[:], scale=-1.0)
                        res = zpool.tile([P, C_OUT], F32, tag="res")
                        nc.scalar.activation(res[:], lg[:], AF.Identity,
                                             bias=nlsn[:])
                        nc.sync.dma_start(out=out_ext[t * P : t * P + rows, :],
                                          in_=res[:rows, :])

        agg_layer(lambda k, kend: T1_ck[k][:, :], t1_tiles, t2_tiles, 1)
        nc.gpsimd.collective_compute(
            "AllGather", mybir.AluOpType.bypass,
            replica_groups=[list(range(NCORES))],
            ins=[t2_shard[:].opt()], outs=[T2_full[:].opt()],
        )
        agg_layer(lambda k, kend: T2_full[k * CH : kend, :], t2_tiles, None, 2)

    nc.finalize()
    return nc


_CACHE = {}


def kernel(**inputs):
    in_maps, meta = _prep_host(
        inputs["x"], inputs["edge_index"], inputs["W0"], inputs["b0"],
        inputs["W1"], inputs["b1"], inputs["Wl"], inputs["bl"])
    key = (meta["tot_blocks"], meta["tot_sched"],
           meta["nblk_r"].tobytes(),
           str(meta["sched"]).__hash__())
    if key not in _CACHE:
        _CACHE[key] = _build_program(meta)
    nc = _CACHE[key]
    trace = bool(int(__import__("os").environ.get("KERNEL_TRACE", "0")))
    res = run_bass_kernel_spmd(nc, in_maps, list(range(NCORES)), trace=trace)
    kernel.last_results = res
    out = np.concatenate([res.results[c]["out"] for c in range(NCORES)], axis=0)
    return out.astype(np.float32)
